# revision 73
# baseline (speedup 1.0000x reference)
"""BottleneckAttention TRN2 kernel: 8 NeuronCores, one (batch, head) pair per core.

Decomposition (per core, batch b / head i):
  q = (scale * Wq_i) @ x_b          [64, 4096]   (d-major)
  k = Wk_i @ x_b                    [64, 4096]
  vT = (Wv_i @ x_b)^T               [4096, 64]   (n-major, built chunkwise)
  Height rel-bias folded into the score matmul via an augmented contraction:
     K_aug = [k; Ih]  Q_aug = [q; RH^T]     (Ih[h',j] = 1 if j//64 == h')
     S^T[j,q] = K_aug^T Q_aug = content + height-bias
  Width rel-bias applied multiplicatively after exp (separability of exp):
     E = exp(S^T) * ew_dup[jw(j), q],  ew = exp(RW^T)
  PV + row-sums fused: vT_aug = [vT | 1] so out rows 0..63 = unnormalized
  attention output (transposed [d, q]), row 64 = softmax denominators.
  Output projection partial: P = Wout[:, i*64:(i+1)*64] @ out, then columns
  scaled by 1/sums (normalization commutes with the d-contraction).
Host sums the 4 per-head partials per batch and adds the residual x.

All inputs are pre-converted to bf16 on host (no on-device casts). The
steady-state pipeline is ACT(exp)-bound at ~1.1us per 128-key chunk; the
schedule keeps ACT saturated: PE builds/projections go to a dedicated PSUM
slot, psum->sbuf copies go to DVE, softmax denominators use the fast
approximate reciprocal, ew is exp'd in per-quarter chunks so quarter 0 can
start before the full width table is ready.
"""

import numpy as np
import ml_dtypes

import concourse.bass as bass
import concourse.bacc as bacc
import concourse.tile as tile
from concourse import mybir
from concourse.bass_utils import run_bass_kernel_spmd

F32 = mybir.dt.float32
BF16 = mybir.dt.bfloat16
AF = mybir.ActivationFunctionType

HEADS, B, C, HH, WW = 4, 2, 256, 64, 64
N = HH * WW           # 4096
DH = C // HEADS       # 64
NQ = 4                # query blocks
QB = N // NQ          # 1024 query cols per block
NJC = 32              # key chunks of 128
PVLAG = 6

# Schraudolph fast exp: exp(x) ~= bitcast_f32(int32(A*x + B)), ~4% max err.
# A handful of key chunks per quarter take this DVE path to offload the
# ACT engine (the pipeline bottleneck); the rest use exact ACT exp.
FEXP_A = 12102203.161561485     # 2^23 / ln 2
FEXP_B = float((127 << 23) - 486411)
FAST_JC = ()                    # DVE fast-exp: off (the TS+TT pair in the DVE
                                # FIFO delays e-muls -> PV stalls; net loss)
GPS_JC = ()                     # GpSimd e-muls: off (its SBUF port is shared
                                # with VectorE; measured 2.9us/mul, poisons DVE)


def _body(tc, io):
    from contextlib import ExitStack
    with ExitStack() as ctx:
        _body_inner(tc, io, ctx)


def _body_inner(tc, io, ctx):
    nc = tc.nc
    xb, wq, wk, wv, wo, relw, relh, ih, out = (
        io["xb"], io["wq"], io["wk"], io["wv"], io["wo"],
        io["relw"], io["relh"], io["ih"], io["out"],
    )

    big = ctx.enter_context(tc.tile_pool(name="big", bufs=1))
    rot = ctx.enter_context(tc.tile_pool(name="rot", bufs=16))
    fpool = ctx.enter_context(tc.tile_pool(name="fpool", bufs=3))
    ep = ctx.enter_context(tc.tile_pool(name="ep", bufs=2))
    spool = ctx.enter_context(tc.tile_pool(name="spool", bufs=2, space="PSUM"))
    opool = ctx.enter_context(tc.tile_pool(name="opool", bufs=1, space="PSUM"))
    ipool = ctx.enter_context(tc.tile_pool(name="ipool", bufs=1, space="PSUM"))
    dpool = ctx.enter_context(tc.tile_pool(name="dpool", bufs=2, space="DRAM"))

    # ---- SBUF tiles -------------------------------------------------
    xb_bf = big.tile([128, 2, N], BF16)
    wq_bf = big.tile([128, 2, DH], BF16)
    wk_bf = big.tile([128, 2, DH], BF16)
    wv_bf = big.tile([128, 2, DH], BF16)
    wo_bf = big.tile([64, 256], BF16)
    relw_bf = big.tile([64, 127], BF16)
    relh_bf = big.tile([64, 127], BF16)
    K_aug = big.tile([128, N], BF16)
    Q_aug = big.tile([128, N], BF16)
    ew_dup = big.tile([128, N], BF16)
    rwt = big.tile([64, N], BF16)
    vt_aug = big.tile([128, NJC, 65], BF16)
    h_sb = big.tile([64, N], BF16)

    # ---- input DMAs ------------------------------------------------
    # First-needed data first; weight loads go on the gpsimd queue so the
    # sync queue's issue cost doesn't delay xb quarter 0.
    # xb pieces fan out over four queue engines (all idle this early) so the
    # transfers overlap instead of arriving ~2.3us apart off two queues.
    # One DMA per xb quarter (each fans out across all 16 DMA engines on its
    # own), spread over the three DMA-capable issue queues so all four are
    # in flight almost immediately.
    xv = xb.rearrange("(cc p) n -> p cc n", p=128)
    nc.gpsimd.dma_start(out=wq_bf, in_=wq.rearrange("(cc p) d -> p cc d", p=128))
    nc.sync.dma_start(out=xb_bf[:, :, 0:QB], in_=xv[:, :, 0:QB])
    nc.scalar.dma_start(out=xb_bf[:, :, bass.ts(1, QB)], in_=xv[:, :, bass.ts(1, QB)])
    for t_bf, t_d in ((wk_bf, wk), (wv_bf, wv)):
        nc.gpsimd.dma_start(out=t_bf, in_=t_d.rearrange("(cc p) d -> p cc d", p=128))
    nc.sync.dma_start(out=xb_bf[:, :, bass.ts(2, QB)], in_=xv[:, :, bass.ts(2, QB)])
    nc.scalar.dma_start(out=xb_bf[:, :, bass.ts(3, QB)], in_=xv[:, :, bass.ts(3, QB)])
    nc.sync.dma_start(out=relh_bf, in_=relh)
    nc.scalar.dma_start(out=relw_bf, in_=relw)
    # Ih rows of K_aug straight from dram (bf16, exact 0/1)
    nc.sync.dma_start(out=K_aug[64:128, :], in_=ih)
    nc.gpsimd.dma_start(out=wo_bf, in_=wo)

    nc.gpsimd.memset(vt_aug[:, :, 64:65], 1.0)
    ones_row = big.tile([1, 128], BF16)
    nc.gpsimd.memset(ones_row, 1.0)

    # PE warm-up while the first xb quarter lands.
    warm = big.tile([128, 512], BF16)
    nc.vector.memset(warm, 0.0)
    for _ in range(11):
        wps = spool.tile([128, 512], F32, tag="sp")
        nc.tensor.matmul(wps, warm[:, 0:128], warm, start=True, stop=True)

    # ---- build helpers ---------------------------------------------
    # psum->sbuf copies: ACT while it is idle (prologue), DVE in-loop.
    def _copy(eng, dst, src):
        if eng == "act":
            nc.scalar.activation(out=dst, in_=src, func=AF.Copy)
        else:
            nc.vector.tensor_copy(out=dst, in_=src)

    def qk_build(dst, w_bf, qq, pool, tag, eng="dve"):
        ps = pool.tile([128, QB], F32, tag=tag)
        for cc in range(2):
            for h in range(2):
                nc.tensor.matmul(
                    ps[0:64, bass.ts(h, 512)],
                    w_bf[:, cc, :],
                    xb_bf[:, cc, qq * QB + h * 512: qq * QB + (h + 1) * 512],
                    start=(cc == 0), stop=(cc == 1),
                )
        _copy(eng, dst[0:64, bass.ts(qq, QB)], ps[0:64, :])

    def rh_build(g, pool, tag, eng="dve"):
        # RH^T[jh, n=(x,y)] = sum_d relh[jh - x + 63, d] * q[d, n]
        ps = pool.tile([128, QB], F32, tag=tag)
        for xi in range(16):
            xx = g * 16 + xi
            nc.tensor.matmul(
                ps[0:64, bass.ts(xi, 64)],
                relh_bf[:, 63 - xx: 127 - xx],
                Q_aug[0:64, xx * 64: (xx + 1) * 64],
                start=True, stop=True,
            )
        _copy(eng, Q_aug[64:128, bass.ts(g, QB)], ps[0:64, :])

    q_xy = Q_aug[0:64, :].rearrange("d (x y) -> d x y", y=64)
    rwt_xy = rwt.rearrange("jw (x y) -> jw x y", y=64)

    def rw_build(g, pool, tag, eng="dve"):
        # RW^T[jw, n=(x,y)] = sum_d relw[jw - y + 63, d] * q[d, n]
        ps = pool.tile([128, QB], F32, tag=tag)
        for yi in range(16):
            yy = g * 16 + yi
            nc.tensor.matmul(
                ps[0:64, bass.ts(yi, 64)],
                relw_bf[:, 63 - yy: 127 - yy],
                q_xy[:, :, yy],
                start=True, stop=True,
            )
        # ps free layout is [yi, x]; rwt quarter slice wants [x, y].
        _copy(eng, rwt_xy[:, :, g * 16:(g + 1) * 16],
              ps[0:64, :].rearrange("p (yi x) -> p x yi", x=64))

    def vt_build(g, pool, tag, eng="dve"):
        ps = pool.tile([128, 8, 64], F32, tag=tag)
        for ci in range(8):
            chunk = g * 8 + ci
            for cc in range(2):
                nc.tensor.matmul(
                    ps[:, ci, :],
                    xb_bf[:, cc, chunk * 128: (chunk + 1) * 128],
                    wv_bf[:, cc, :],
                    start=(cc == 0), stop=(cc == 1),
                )
        _copy(eng, vt_aug[:, g * 8: (g + 1) * 8, 0:64], ps)

    def ew_build(c):
        # ew chunk c covers query x-block c; only quarter c's muls need it.
        nc.scalar.activation(out=ew_dup[0:64, bass.ts(c, QB)],
                             in_=rwt[:, bass.ts(c, QB)], func=AF.Exp)
        nc.vector.tensor_copy(out=ew_dup[64:128, bass.ts(c, QB)],
                              in_=ew_dup[0:64, bass.ts(c, QB)])

    # ---- prologue: all Q/K/V/rw builds -----------------------------
    # The main loop's quarter 0 has zero PE slack (S + deferred PV fill it),
    # so every build lives here, where ACT/DVE are otherwise idle. Builds
    # round-robin through 4 psum slots (spool x2 + ipool + opool, all free
    # before the loop) so a build never waits on the previous build's copy;
    # copies alternate between ACT and DVE so neither chain lags.
    slots = [(spool, "sp"), (spool, "sp"), (ipool, "ij"), (opool, "ov")]
    engs = ["act", "dve"]
    # rw right after the Q builds (it gates ew0 -> first e-mul) and ew0
    # emitted immediately after rw, BEFORE the K/V builds: engine FIFOs run
    # in emission order, so a late ew0 would queue behind the K/V copies on
    # ACT and delay every e-mul. Copy engines are assigned so the ACT chain
    # ahead of exp(0) is just {q0,q2,rw0,rw2,ew0,k0,v0}.
    builds = [("q", 0, "act"), ("q", 1, "dve"), ("rh", 0, "dve"),
              ("q", 2, "act"), ("q", 3, "dve"),
              ("rw", 0, "act"), ("rw", 1, "dve"), ("rw", 2, "act"),
              ("rw", 3, "dve"),
              ("ew", 0, None),
              ("k", 0, "act"), ("v", 0, "act"),
              ("k", 1, "dve"), ("v", 1, "dve"), ("k", 2, "dve"),
              ("v", 2, "dve"), ("k", 3, "dve"), ("v", 3, "dve")]
    nslot = 0
    for kind, idx, eng in builds:
        if kind == "ew":
            ew_build(0)
            continue
        pool, tag = slots[nslot % 4]
        nslot += 1
        if kind == "q":
            qk_build(Q_aug, wq_bf, idx, pool, tag, eng)
        elif kind == "k":
            qk_build(K_aug, wk_bf, idx, pool, tag, eng)
        elif kind == "v":
            vt_build(idx, pool, tag, eng)
        elif kind == "rh":
            rh_build(idx, pool, tag, eng)
        else:
            rw_build(idx, pool, tag, eng)

    # ---- main attention loop ---------------------------------------
    # Per chunk: S^T matmul (PE) -> exp (ACT) -> *ew (DVE) -> PV (PE).
    # ACT is the bottleneck engine; everything else is scheduled around it.
    def make_proj(qqp, rbc):
        def proj(oh, pool=ipool, tag="ij"):
            pp = pool.tile([128, QB], F32, tag=tag)
            for h in range(2):
                nc.tensor.matmul(
                    pp[:, bass.ts(h, 512)],
                    wo_bf[:, oh * 128: (oh + 1) * 128],
                    h_sb[:, qqp * QB + h * 512: qqp * QB + (h + 1) * 512],
                    start=True, stop=True,
                )
            osb = ep.tile([128, QB], BF16, tag="osb")
            nc.vector.tensor_mul(osb, pp, rbc)
            eng = nc.sync if oh == 0 else nc.gpsimd
            eng.dma_start(
                out=out[oh * 128: (oh + 1) * 128, qqp * QB: (qqp + 1) * QB],
                in_=osb,
            )
        return proj

    o_ps = None
    proj_prev = None

    def drain_a(qqp, rs0_eng="dve"):
        # The two o_ps readers (denominator row, h rows) -- these must run
        # before the next quarter's PV reuses the single opool slot. At the
        # tail the denominator copy goes on ACT (idle there).
        rs0 = ep.tile([1, QB], F32, tag="rs0")
        _copy(rs0_eng, rs0, o_ps[64:65, :])
        nc.vector.tensor_copy(out=h_sb[:, bass.ts(qqp, QB)], in_=o_ps[0:64, :])
        return rs0

    def drain_b1(rs0):
        # 1/sums in bf16 (first half of the broadcast chain).
        rsb = ep.tile([1, QB], F32, tag="rsb")
        nc.vector.reciprocal_approx_fast(out=rsb, in_=rs0)
        rsb_bf = ep.tile([1, QB], BF16, tag="rsbf")
        nc.vector.tensor_copy(out=rsb_bf, in_=rsb)
        return rsb_bf

    def drain_b2(qqp, rsb_bf, pool, tag):
        # Broadcast to 128 partitions with a PE ones-outer-product. Split
        # from drain_b1 so the DVE work spreads over the quarter instead of
        # front-loading (which starves e-muls and dents the exp stream).
        rbc_ps = pool.tile([128, QB], F32, tag=tag)
        for h in range(2):
            nc.tensor.matmul(rbc_ps[:, bass.ts(h, 512)], ones_row,
                             rsb_bf[:, bass.ts(h, 512)], start=True, stop=True)
        rbc = ep.tile([128, QB], BF16, tag="rbc")
        nc.vector.tensor_copy(out=rbc, in_=rbc_ps)
        return make_proj(qqp, rbc)

    for qq in range(NQ):
        if qq > 0:
            rs0_prev = drain_a(qq - 1)
        o_ps = opool.tile([128, QB], F32, tag="ov")
        e_tiles = [None] * NJC

        e0_tiles = [None] * NJC

        def mul_stage(jc):
            eng = nc.gpsimd if jc in GPS_JC else nc.vector
            e = rot.tile([128, QB], BF16, tag="e")
            eng.tensor_mul(e, e0_tiles[jc], ew_dup[:, bass.ts(qq, QB)])
            e0_tiles[jc] = None
            e_tiles[jc] = e

        def s_stage(jc, do_mul=True):
            ps = spool.tile([128, QB], F32, tag="sp")
            for h in range(2):
                nc.tensor.matmul(
                    ps[:, bass.ts(h, 512)],
                    K_aug[:, jc * 128: (jc + 1) * 128],
                    Q_aug[:, qq * QB + h * 512: qq * QB + (h + 1) * 512],
                    start=True, stop=True,
                )
            if jc in FAST_JC:
                e0f = fpool.tile([128, QB], F32, tag="e0f")
                nc.vector.tensor_scalar(
                    out=e0f.bitcast(mybir.dt.int32), in0=ps,
                    scalar1=FEXP_A, scalar2=FEXP_B,
                    op0=mybir.AluOpType.mult, op1=mybir.AluOpType.add)
                e0_tiles[jc] = e0f
            else:
                e0 = rot.tile([128, QB], BF16, tag="e0")
                nc.scalar.activation(out=e0, in_=ps, func=AF.Exp)
                e0_tiles[jc] = e0
            if do_mul:
                mul_stage(jc)

        def pv_stage(jc):
            for h in range(2):
                nc.tensor.matmul(
                    o_ps[0:65, bass.ts(h, 512)],
                    vt_aug[:, jc, :],
                    e_tiles[jc][:, bass.ts(h, 512)],
                    start=(jc == 0), stop=(jc == NJC - 1),
                )
            e_tiles[jc] = None

        for t in range(NJC + PVLAG):
            if t < NJC:
                s_stage(t)
            if qq > 0:
                if t == 8:
                    proj_prev = drain_b2(qq - 1, drain_b1(rs0_prev),
                                         ipool, "ij")
                elif t == 18:
                    proj_prev(0)
                elif t == 22:
                    proj_prev(1)
            if qq < NQ - 1:
                if t == 4:
                    rh_build(qq + 1, ipool, "ij")
                elif t == 24:
                    ew_build(qq + 1)
            if t >= PVLAG:
                pv_stage(t - PVLAG)

    # final quarter epilogue: projections on free S-pool slots so the two
    # output halves run in parallel instead of serializing through ipool.
    rs0_last = drain_a(NQ - 1, rs0_eng="act")
    proj_last = drain_b2(NQ - 1, drain_b1(rs0_last), spool, "sp")
    proj_last(0, pool=spool, tag="sp")
    proj_last(1, pool=spool, tag="sp")


_NC_CACHE = {}


def _build():
    if "nc" in _NC_CACHE:
        return _NC_CACHE["nc"]
    nc = bacc.Bacc("TRN2", target_bir_lowering=False, debug=False, num_devices=8)
    io = {
        "xb": nc.dram_tensor("xb", [C, N], BF16, kind="ExternalInput").ap(),
        "wq": nc.dram_tensor("wq", [C, DH], BF16, kind="ExternalInput").ap(),
        "wk": nc.dram_tensor("wk", [C, DH], BF16, kind="ExternalInput").ap(),
        "wv": nc.dram_tensor("wv", [C, DH], BF16, kind="ExternalInput").ap(),
        "wo": nc.dram_tensor("wo", [DH, C], BF16, kind="ExternalInput").ap(),
        "relw": nc.dram_tensor("relw", [DH, 127], BF16, kind="ExternalInput").ap(),
        "relh": nc.dram_tensor("relh", [DH, 127], BF16, kind="ExternalInput").ap(),
        "ih": nc.dram_tensor("ih", [64, N], BF16, kind="ExternalInput").ap(),
        "out": nc.dram_tensor("out", [C, N], BF16, kind="ExternalOutput").ap(),
    }
    with tile.TileContext(nc) as tc:
        _body(tc, io)
    nc.compile()
    _NC_CACHE["nc"] = nc
    return nc


_last_in_maps = None


def kernel(x, w_qkv, w_out, rel_height, rel_width):
    global _last_in_maps
    bf16 = ml_dtypes.bfloat16
    x = np.ascontiguousarray(np.asarray(x, np.float32))
    w_qkv = np.asarray(w_qkv, np.float32)
    w_out = np.asarray(w_out, np.float32)
    rel_height = np.asarray(rel_height, np.float32)
    rel_width = np.asarray(rel_width, np.float32)

    scale = np.float32(DH ** -0.5)
    ih_const = np.ascontiguousarray(
        np.repeat(np.eye(64, dtype=np.float32), 64, axis=1).astype(bf16))
    relw_t = np.ascontiguousarray(rel_width.T.astype(bf16))
    relh_t = np.ascontiguousarray(rel_height.T.astype(bf16))

    xb_bf = [np.ascontiguousarray(x[b].reshape(C, N).astype(bf16)) for b in range(B)]

    in_maps = []
    for g in range(8):
        b, i = divmod(g, HEADS)
        sl = slice(i * DH, (i + 1) * DH)
        in_maps.append({
            "xb": xb_bf[b],
            "wq": np.ascontiguousarray((w_qkv[i * DH:(i + 1) * DH] * scale).T.astype(bf16)),
            "wk": np.ascontiguousarray(w_qkv[C + i * DH: C + (i + 1) * DH].T.astype(bf16)),
            "wv": np.ascontiguousarray(w_qkv[2 * C + i * DH: 2 * C + (i + 1) * DH].T.astype(bf16)),
            "wo": np.ascontiguousarray(w_out[:, sl].T.astype(bf16)),
            "relw": relw_t,
            "relh": relh_t,
            "ih": ih_const,
        })

    _last_in_maps = in_maps
    nc = _build()
    res = run_bass_kernel_spmd(nc, in_maps, core_ids=list(range(8)))
    parts = [np.asarray(r["out"]).astype(np.float32) for r in res.results]
    outf = np.empty((B, C, N), np.float32)
    for b in range(B):
        outf[b] = parts[4 * b] + parts[4 * b + 1] + parts[4 * b + 2] + parts[4 * b + 3]
        outf[b] += x[b].reshape(C, N)
    return outf.reshape(B, C, HH, WW)
